# revision 3
# baseline (speedup 1.0000x reference)
"""Trainium2 Bass kernel for nn_Curv_Net (masked-MLP / GNN message passing).

Strategy: data-parallel over the batch dim across 8 NeuronCores (256 rows
each).  All weights are pre-masked (W*mask), transposed to [K, M], cast to
bf16 and packed into contiguous 1 MiB DMA chunks on the host.  On-device
everything flows in a transposed activation layout actT[feature, batch]:
each dense layer computes outT = (W*mask) @ actT via PE matmuls with the
weight tile stationary ([K=128, M=128]) and the activation tile moving
([K=128, N=256]), accumulating K in PSUM, then applies sigmoid (+bias) on
the scalar engine and the mix/scale ops on the vector engine.  The final
mean-centering is folded into W7 on the host: (lp - mean(lp)) @ W7.T ==
lp @ (W7 - sum(W7)/OUT).T exactly.
"""

import numpy as np
import ml_dtypes

B, IN, ED, PW, OUT, CL, NK = 2048, 4096, 8192, 2048, 256, 16, 32
NCORES = 8
BC = B // NCORES  # 256 batch rows per core

BF = ml_dtypes.bfloat16
F32 = np.float32

TRACE = False
TRACE_DIR = None

_prog_cache = {}


def _pack_w(wT_bf, mgw):
    """wT_bf [K, M] bf16 -> [MGn*KCn, 128, 8, mgw] chunk-contiguous.

    chunk (mg, kc) holds rows kc*1024..+1024, cols mg*mgw..+mgw with layout
    [p, t, m] = wT[kc*1024 + t*128 + p, mg*mgw + m].
    """
    K, M = wT_bf.shape
    KCn = K // 1024
    MGn = M // mgw
    a = wT_bf.reshape(KCn, 8, 128, MGn, mgw).transpose(3, 0, 2, 1, 4)
    return np.ascontiguousarray(a).reshape(MGn * KCn, 128, 8, mgw)


def _pack_act(xT, dtype):
    """xT [K, BC] -> [128, K/128, BC] p-major contiguous."""
    K = xT.shape[0]
    a = xT.reshape(K // 128, 128, xT.shape[1]).transpose(1, 0, 2)
    return np.ascontiguousarray(a).astype(dtype)


def _pack_vec(v):
    """v [n] -> [128, n/128] f32."""
    return np.ascontiguousarray(v.reshape(-1, 128).T).astype(F32)


def _pack_mask(m):
    """mask [K, NK] -> [128, K/128, NK] bf16 p-major."""
    K = m.shape[0]
    a = m.reshape(K // 128, 128, NK).transpose(1, 0, 2)
    return np.ascontiguousarray(a.astype(BF))


def _build_program():
    if "nc" in _prog_cache:
        return _prog_cache["nc"]

    import concourse.bacc as bacc
    import concourse.mybir as mybir
    import concourse.tile as tile
    from concourse.alu_op_type import AluOpType

    bf16 = mybir.dt.bfloat16
    f32 = mybir.dt.float32
    SIG = mybir.ActivationFunctionType.Sigmoid

    nc = bacc.Bacc("TRN2", target_bir_lowering=False, debug=False)

    # ---- DRAM I/O -------------------------------------------------------
    d = {}
    d["xg"] = nc.dram_tensor("xg", [128, IN // 128, BC], bf16, kind="ExternalInput")
    d["iv"] = nc.dram_tensor("iv", [128, IN // 128, BC], bf16, kind="ExternalInput")
    d["cv"] = nc.dram_tensor("cv", [128, ED // 128, BC], bf16, kind="ExternalInput")
    d["cl"] = nc.dram_tensor("cl", [CL, BC], bf16, kind="ExternalInput")
    d["w1p"] = nc.dram_tensor("w1p", [32, 128, 8, 512], bf16, kind="ExternalInput")
    d["w2p"] = nc.dram_tensor("w2p", [64, 128, 8, 512], bf16, kind="ExternalInput")
    d["w3p"] = nc.dram_tensor("w3p", [32, 128, 8, 512], bf16, kind="ExternalInput")
    d["w4p"] = nc.dram_tensor("w4p", [2, 128, 8, 256], bf16, kind="ExternalInput")
    d["w5t"] = nc.dram_tensor("w5t", [128, 2, OUT], bf16, kind="ExternalInput")
    d["w6a"] = nc.dram_tensor("w6a", [128, 3, OUT], bf16, kind="ExternalInput")
    d["w6b"] = nc.dram_tensor("w6b", [CL, OUT], bf16, kind="ExternalInput")
    d["w7ct"] = nc.dram_tensor("w7ct", [128, 2], f32, kind="ExternalInput")
    for name, n in [("b1t", 32), ("a1t", 32), ("c1t", 32),
                    ("b2t", 64), ("a2t", 64), ("c2t", 64),
                    ("b3t", 16), ("mp3t", 16), ("b4t", 2), ("b5t", 2)]:
        d[name] = nc.dram_tensor(name, [128, n], f32, kind="ExternalInput")
    d["gmp"] = nc.dram_tensor("gmp", [128, 32, NK], bf16, kind="ExternalInput")
    d["imp"] = nc.dram_tensor("imp", [128, 32, NK], bf16, kind="ExternalInput")
    d["cmp"] = nc.dram_tensor("cmp", [128, 64, NK], bf16, kind="ExternalInput")
    d["pmp"] = nc.dram_tensor("pmp", [128, 16, NK], bf16, kind="ExternalInput")
    yd = nc.dram_tensor("y", [1, BC], f32, kind="ExternalOutput")

    with tile.TileContext(nc) as tc:
        with (
            tc.tile_pool(name="const", bufs=1) as cpool,
            tc.tile_pool(name="wstream", bufs=10) as wpool,
            tc.tile_pool(name="fwork", bufs=3) as fpool,
            tc.tile_pool(name="mixin", bufs=8) as ivpool,
            tc.tile_pool(name="psum_mm", bufs=5, space="PSUM") as ppool,
            tc.tile_pool(name="psum_sm", bufs=2, space="PSUM") as spool,
        ):
            def cload(name, shape, dtype):
                t = cpool.tile(shape, dtype, tag=name)
                nc.sync.dma_start(t[:], d[name][:])
                return t

            # persistent constants
            act1 = cload("xg", [128, 32, BC], bf16)
            cl_t = cload("cl", [CL, BC], bf16)
            gm = cload("gmp", [128, 32, NK], bf16)
            im = cload("imp", [128, 32, NK], bf16)
            cm = cload("cmp", [128, 64, NK], bf16)
            pm = cload("pmp", [128, 16, NK], bf16)
            w5t = cload("w5t", [128, 2, OUT], bf16)
            w6a = cload("w6a", [128, 3, OUT], bf16)
            w6b = cload("w6b", [CL, OUT], bf16)
            w7t = cload("w7ct", [128, 2], f32)
            vt = {}
            for name, n in [("b1t", 32), ("a1t", 32), ("c1t", 32),
                            ("b2t", 64), ("a2t", 64), ("c2t", 64),
                            ("b3t", 16), ("mp3t", 16), ("b4t", 2), ("b5t", 2)]:
                vt[name] = cload(name, [128, n], f32)

            act2 = cpool.tile([128, 32, BC], bf16, tag="act2")
            act3 = cpool.tile([128, 64, BC], bf16, tag="act3")
            act4 = cpool.tile([128, 16, BC], bf16, tag="act4")
            act5 = cpool.tile([128, 2, BC], bf16, tag="act5")
            act6 = cpool.tile([128, 2, BC], bf16, tag="act6")
            lp_t = cpool.tile([128, 2, BC], f32, tag="lp")
            t2 = cpool.tile([128, BC], bf16, tag="t2")

            def dense_layer(wdram, K_kt, mgw, MGn, act_in, post):
                """outT m-tile groups of mgw cols; calls post(m, psum)."""
                jw = mgw // 128
                KCn = K_kt // 8
                for mg in range(MGn):
                    chunks = []
                    for kc in range(KCn):
                        wt = wpool.tile([128, 8, mgw], bf16, tag="wt")
                        nc.sync.dma_start(wt[:], wdram[mg * KCn + kc])
                        chunks.append(wt)
                    for j in range(jw):
                        ps = ppool.tile([128, BC], f32, tag="ps")
                        for kt in range(K_kt):
                            nc.tensor.matmul(
                                ps[:],
                                chunks[kt // 8][:, kt % 8, j * 128:(j + 1) * 128],
                                act_in[:, kt, :],
                                start=(kt == 0),
                                stop=(kt == K_kt - 1),
                            )
                        post(mg * jw + j, ps)

            def kept(mask_t, K_kt, act_in, row0):
                kp = spool.tile([128, BC], f32, tag="kp")
                for kt in range(K_kt):
                    nc.tensor.matmul(kp[0:NK, :], mask_t[:, kt, :], act_in[:, kt, :],
                                     start=(kt == 0), stop=(kt == K_kt - 1))
                nc.scalar.copy(t2[row0:row0 + NK, :], kp[0:NK, :])

            def mix_post(bias, avec, cvec, mixd, act_out):
                def post(m, ps):
                    x1f = fpool.tile([128, BC], f32, tag="x1f")
                    nc.scalar.activation(x1f[:], ps[:], SIG, bias=bias[:, m:m + 1])
                    mx = ivpool.tile([128, BC], bf16, tag="mx")
                    nc.sync.dma_start(mx[:], mixd[:, m, :])
                    tmp = fpool.tile([128, BC], f32, tag="tmp")
                    nc.vector.tensor_scalar_mul(tmp[:], mx[:], avec[:, m:m + 1])
                    nc.vector.scalar_tensor_tensor(
                        act_out[:, m, :], x1f[:], cvec[:, m:m + 1], tmp[:],
                        AluOpType.mult, AluOpType.add)
                return post

            # ---- layer 1: [IN] -> [IN], mix with x_invmea ----
            dense_layer(d["w1p"], 32, 512, 8, act1,
                        mix_post(vt["b1t"], vt["a1t"], vt["c1t"], d["iv"], act2))
            kept(gm, 32, act1, 0)
            kept(im, 32, act2, NK)

            # ---- layer 2: [IN] -> [ED], mix with x_curv ----
            dense_layer(d["w2p"], 32, 512, 16, act2,
                        mix_post(vt["b2t"], vt["a2t"], vt["c2t"], d["cv"], act3))
            kept(cm, 64, act3, 2 * NK)

            # ---- layer 3: [ED] -> [PW], scale by mp3 ----
            def post3(m, ps):
                x1f = fpool.tile([128, BC], f32, tag="x1f")
                nc.scalar.activation(x1f[:], ps[:], SIG, bias=vt["b3t"][:, m:m + 1])
                nc.vector.tensor_scalar_mul(act4[:, m, :], x1f[:],
                                            vt["mp3t"][:, m:m + 1])
            dense_layer(d["w3p"], 64, 512, 4, act3, post3)
            kept(pm, 16, act4, 3 * NK)

            # ---- layer 4: [PW] -> [OUT] ----
            def post4(m, ps):
                nc.scalar.activation(act5[:, m, :], ps[:], SIG,
                                     bias=vt["b4t"][:, m:m + 1])
            dense_layer(d["w4p"], 16, 256, 1, act4, post4)

            # ---- layer 5: [OUT] -> [OUT] ----
            for j in range(2):
                ps = ppool.tile([128, BC], f32, tag="ps")
                for kt in range(2):
                    nc.tensor.matmul(ps[:], w5t[:, kt, j * 128:(j + 1) * 128],
                                     act5[:, kt, :], start=(kt == 0), stop=(kt == 1))
                nc.scalar.activation(act6[:, j, :], ps[:], SIG,
                                     bias=vt["b5t"][:, j:j + 1])

            # ---- layer 6: x_cat [400] -> lp [OUT] ----
            for j in range(2):
                jc = slice(j * 128, (j + 1) * 128)
                ps = ppool.tile([128, BC], f32, tag="ps")
                nc.tensor.matmul(ps[:], w6a[:, 0, jc], act6[:, 0, :],
                                 start=True, stop=False)
                nc.tensor.matmul(ps[:], w6a[:, 1, jc], act6[:, 1, :],
                                 start=False, stop=False)
                nc.tensor.matmul(ps[:], w6a[:, 2, jc], t2[:],
                                 start=False, stop=False)
                nc.tensor.matmul(ps[:], w6b[:, jc], cl_t[:],
                                 start=False, stop=True)
                nc.scalar.activation(lp_t[:, j, :], ps[:], SIG)

            # ---- final: out = w7c @ lp (fp32, mean-centering folded in) ----
            fps = spool.tile([128, BC], f32, tag="kp")
            nc.tensor.matmul(fps[0:1, :], w7t[:, 0:1], lp_t[:, 0, :],
                             start=True, stop=False)
            nc.tensor.matmul(fps[0:1, :], w7t[:, 1:2], lp_t[:, 1, :],
                             start=False, stop=True)
            osb = cpool.tile([1, BC], f32, tag="osb")
            nc.scalar.copy(osb[:], fps[0:1, :])
            nc.sync.dma_start(yd[:], osb[:])

    nc.compile()
    _prog_cache["nc"] = nc
    return nc


def _host_prep(inputs):
    """Shared (weight) arrays, identical for every core."""
    W1, W2, W3 = inputs["W1"], inputs["W2"], inputs["W3"]
    m1t = np.ascontiguousarray((W1 * inputs["Adj"]).T).astype(BF)
    m2t = np.ascontiguousarray((W2 * inputs["edge_mask"]).T).astype(BF)
    m3t = np.ascontiguousarray((W3 * inputs["pathway_mask"]).T).astype(BF)
    w4t = np.ascontiguousarray(inputs["W4"].T).astype(BF)
    w5T = np.ascontiguousarray(inputs["W5"].T).astype(BF)
    w6T = np.ascontiguousarray(inputs["W6"].T).astype(BF)  # [400, 256]
    w7c = (inputs["W7"][0] - inputs["W7"].sum() / OUT).astype(F32)

    shared = {
        "w1p": _pack_w(m1t, 512),
        "w2p": _pack_w(m2t, 512),
        "w3p": _pack_w(m3t, 512),
        "w4p": _pack_w(w4t, 256),
        "w5t": np.ascontiguousarray(w5T.reshape(2, 128, OUT).transpose(1, 0, 2)),
        "w6a": np.ascontiguousarray(w6T[:384].reshape(3, 128, OUT).transpose(1, 0, 2)),
        "w6b": np.ascontiguousarray(w6T[384:400]),
        "w7ct": _pack_vec(w7c),
        "b1t": _pack_vec(inputs["b1"]),
        "a1t": _pack_vec(inputs["mp11"] * inputs["mp1"]),
        "c1t": _pack_vec(inputs["mp12"] * inputs["mp1"]),
        "b2t": _pack_vec(inputs["b2"]),
        "a2t": _pack_vec(inputs["mp21"] * inputs["mp2"]),
        "c2t": _pack_vec(inputs["mp22"] * inputs["mp2"]),
        "b3t": _pack_vec(inputs["b3"]),
        "mp3t": _pack_vec(inputs["mp3"]),
        "b4t": _pack_vec(inputs["b4"]),
        "b5t": _pack_vec(inputs["b5"]),
        "gmp": _pack_mask(inputs["top_gene_mask"]),
        "imp": _pack_mask(inputs["top_invmea_mask"]),
        "cmp": _pack_mask(inputs["top_curv_mask"]),
        "pmp": _pack_mask(inputs["top_path_mask"]),
    }
    return shared


def kernel(**inputs):
    inputs = {k: np.asarray(v) for k, v in inputs.items()}
    nc = _build_program()
    shared = _host_prep(inputs)

    in_maps = []
    for c in range(NCORES):
        s = slice(c * BC, (c + 1) * BC)
        m = dict(shared)
        m["xg"] = _pack_act(inputs["x_gene"][s].T.astype(BF), BF)
        m["iv"] = _pack_act(inputs["x_invmea"][s].T.astype(BF), BF)
        m["cv"] = _pack_act(inputs["x_curv"][s].T.astype(BF), BF)
        m["cl"] = np.ascontiguousarray(inputs["clinn"][s].T).astype(BF)
        in_maps.append(m)

    from concourse.bass_utils import run_bass_kernel_spmd

    kwargs = {}
    if TRACE:
        import sys, types
        try:
            from trn_agent_boot.trn_boot import _ntff_profile_via_ctypes
            hook = _ntff_profile_via_ctypes("/opt/axon/libaxon_pjrt.so")
            if hook is not None:
                mod = types.ModuleType("antenv.axon_hooks")
                mod.get_axon_ntff_profile_hook = lambda: hook
                sys.modules["antenv.axon_hooks"] = mod
                import concourse.bass_utils as _bu
                _bu.upload_artifacts = lambda tmpdir: "local://" + tmpdir
                kwargs["trace"] = True
                if TRACE_DIR:
                    kwargs["tmpdir"] = TRACE_DIR
        except Exception as e:
            print("trace setup failed:", e)

    res = run_bass_kernel_spmd(nc, in_maps, core_ids=list(range(NCORES)), **kwargs)
    if TRACE:
        kernel.last_exec_time_ns = res.exec_time_ns

    out = np.concatenate(
        [res.results[c]["y"].reshape(BC, 1) for c in range(NCORES)], axis=0
    )
    return out.astype(F32)


# revision 13
# speedup vs baseline: 1.5777x; 1.5777x over previous
"""Trainium2 Bass kernel for nn_Curv_Net (masked-MLP / GNN message passing).

Sharding: data-parallel over the batch dim across 8 NeuronCores (256 rows
each).  All masked weights (W*mask) are prepared on the host: transposed to
[K, M], row-normalized and cast to fp8-e4m3 when that is exact (it is for
the reference's constant-fill W1/W2/W3: the masked weight is scale*mask),
otherwise bf16.  On device everything flows in a transposed activation
layout actT[feature, batch]; each dense layer runs PE matmuls with the
weight tile stationary and the activation tile moving (N=256), accumulating
K in PSUM.  The three big layers use fp8 DoubleRow (2 contraction rows per
cycle -> 2x PE throughput); the per-row weight scale is folded into the
sigmoid's scale operand.  The stop-gradient "kept" bypass values are kept
at full precision: kept_gene is computed on the host (pure input
selection), kept_invmea/kept_curv are row-gathered by DMA from the f32
mixed activations before the fp8 cast, and kept_path stays on the bf16
path.  The final mean-centering is folded into W7 on the host:
(lp - mean(lp)) @ W7.T == lp @ (W7 - sum(W7)/OUT).T exactly.
"""

import numpy as np
import ml_dtypes

B, IN, ED, PW, OUT, CL, NK = 2048, 4096, 8192, 2048, 256, 16, 32
NCORES = 8
BC = B // NCORES  # 256 batch rows per core

BF = ml_dtypes.bfloat16
F8 = ml_dtypes.float8_e4m3
F32 = np.float32

TRACE = False
TRACE_DIR = None

_prog_cache = {}


def _pack_w(wT, mgw, sub):
    """wT [K, M] -> [MGn*KCn, 128, sub, mgw] chunk-contiguous.

    chunk (mg, kc) holds rows kc*sub*128..+sub*128, cols mg*mgw..+mgw with
    layout [p, t, m] = wT[kc*sub*128 + t*128 + p, mg*mgw + m].
    """
    K, M = wT.shape
    KCn = K // (sub * 128)
    MGn = M // mgw
    a = wT.reshape(KCn, sub, 128, MGn, mgw).transpose(3, 0, 2, 1, 4)
    return np.ascontiguousarray(a).reshape(MGn * KCn, 128, sub, mgw)


def _pack_act(xT, dtype):
    """xT [K, BC] -> [128, K/128, BC] p-major contiguous."""
    K = xT.shape[0]
    a = xT.reshape(K // 128, 128, xT.shape[1]).transpose(1, 0, 2)
    return np.ascontiguousarray(a).astype(dtype)


def _pack_vec(v):
    """v [n] -> [128, n/128] f32."""
    return np.ascontiguousarray(np.asarray(v, F32).reshape(-1, 128).T).astype(F32)


def _pack_mask(m):
    """mask [K, NK] -> [128, K/128, NK] bf16 p-major."""
    K = m.shape[0]
    a = m.reshape(K // 128, 128, NK).transpose(1, 0, 2)
    return np.ascontiguousarray(a.astype(BF))


def _rowscale_fp8(masked):
    """masked [M, K] -> (scale [M], q [K, M] fp8) with masked == s*q exact,
    or (None, None) if not exactly representable."""
    s = np.abs(masked).max(axis=1)
    s[s == 0] = 1.0
    q = masked / s[:, None]
    q8 = q.astype(F8)
    if not np.array_equal(q8.astype(F32), q):
        return None, None
    return s.astype(F32), np.ascontiguousarray(q8.T)


def _onehot_idx(mask):
    """mask [K, NK] -> row index per column if exactly one-hot, else None."""
    if not np.all((mask == 0) | (mask == 1)):
        return None
    if not np.array_equal(mask.sum(axis=0), np.ones(mask.shape[1], F32)):
        return None
    return np.argmax(mask, axis=0)


def _build_program(mode, iidx=None, cidx=None):
    key = (mode, None if iidx is None else (tuple(iidx), tuple(cidx)))
    if key in _prog_cache:
        return _prog_cache[key]

    import concourse.bacc as bacc
    import concourse.mybir as mybir
    import concourse.tile as tile
    from concourse.alu_op_type import AluOpType

    bf16 = mybir.dt.bfloat16
    fp8 = mybir.dt.float8e4
    f32 = mybir.dt.float32
    SIG = mybir.ActivationFunctionType.Sigmoid
    DR = mybir.MatmulPerfMode.DoubleRow
    fast = mode == "fast"
    adt = fp8 if fast else bf16           # dtype of the big-layer activations
    wsub = 16 if fast else 8              # k-subtiles per big-layer chunk

    nc = bacc.Bacc("TRN2", target_bir_lowering=False, debug=False)

    # ---- DRAM I/O -------------------------------------------------------
    d = {}
    d["xg"] = nc.dram_tensor("xg", [128, IN // 128, BC], adt, kind="ExternalInput")
    d["iv"] = nc.dram_tensor("iv", [128, IN // 128, BC], bf16, kind="ExternalInput")
    d["cv"] = nc.dram_tensor("cv", [128, ED // 128, BC], bf16, kind="ExternalInput")
    d["cl"] = nc.dram_tensor("cl", [CL, BC], bf16, kind="ExternalInput")
    d["w1p"] = nc.dram_tensor("w1p", [(IN // (wsub * 128)) * (IN // 512), 128, wsub, 512], adt, kind="ExternalInput")
    d["w2p"] = nc.dram_tensor("w2p", [(IN // (wsub * 128)) * (ED // 512), 128, wsub, 512], adt, kind="ExternalInput")
    d["w3p"] = nc.dram_tensor("w3p", [(ED // (wsub * 128)) * (PW // 512), 128, wsub, 512], adt, kind="ExternalInput")
    d["w4p"] = nc.dram_tensor("w4p", [2, 128, 8, 256], bf16, kind="ExternalInput")
    d["w5t"] = nc.dram_tensor("w5t", [128, 2, OUT], bf16, kind="ExternalInput")
    d["w6a"] = nc.dram_tensor("w6a", [128, 3, OUT], bf16, kind="ExternalInput")
    d["w6b"] = nc.dram_tensor("w6b", [CL, OUT], bf16, kind="ExternalInput")
    d["w7ct"] = nc.dram_tensor("w7ct", [128, 2], f32, kind="ExternalInput")
    vec_specs = [("b1t", 32), ("a1t", 32), ("c1t", 32),
                 ("b2t", 64), ("a2t", 64), ("c2t", 64),
                 ("b3t", 16), ("mp3t", 16), ("b4t", 2), ("b5t", 2)]
    if fast:
        vec_specs += [("s1t", 32), ("s2t", 64), ("s3t", 16)]
    for name, n in vec_specs:
        d[name] = nc.dram_tensor(name, [128, n], f32, kind="ExternalInput")
    d["pmp"] = nc.dram_tensor("pmp", [128, 16, NK], bf16, kind="ExternalInput")
    d["imp"] = nc.dram_tensor("imp", [128, 32, NK], bf16, kind="ExternalInput")
    d["cmp"] = nc.dram_tensor("cmp", [128, 64, NK], bf16, kind="ExternalInput")
    if fast:
        d["kg"] = nc.dram_tensor("kg", [NK, BC], bf16, kind="ExternalInput")
    else:
        d["gmp"] = nc.dram_tensor("gmp", [128, 32, NK], bf16, kind="ExternalInput")
    yd = nc.dram_tensor("y", [1, BC], f32, kind="ExternalOutput")

    # k-tiles containing at least one kept-selection row
    inv_kts = sorted({idx // 128 for idx in iidx}) if fast else []
    curv_kts = sorted({idx // 128 for idx in cidx}) if fast else []

    with tile.TileContext(nc) as tc:
        with (
            tc.tile_pool(name="const", bufs=1) as cpool,
            tc.tile_pool(name="wstream", bufs=10) as wpool,
            tc.tile_pool(name="fwork", bufs=4) as fpool,
            tc.tile_pool(name="mixin", bufs=8) as ivpool,
            tc.tile_pool(name="psum_mm", bufs=5, space="PSUM") as ppool,
            tc.tile_pool(name="psum_sm", bufs=2, space="PSUM") as spool,
        ):
            def cload(name, shape, dtype):
                t = cpool.tile(shape, dtype, tag=name, name=name + "_sb")
                nc.sync.dma_start(t[:], d[name][:])
                return t

            act1 = cload("xg", [128, 32, BC], adt)
            cl_t = cload("cl", [CL, BC], bf16)
            pm = cload("pmp", [128, 16, NK], bf16)
            w5t = cload("w5t", [128, 2, OUT], bf16)
            w6a = cload("w6a", [128, 3, OUT], bf16)
            w6b = cload("w6b", [CL, OUT], bf16)
            w7t = cload("w7ct", [128, 2], f32)
            vt = {}
            for name, n in vec_specs:
                vt[name] = cload(name, [128, n], f32)

            act2 = cpool.tile([128, 32, BC], adt, tag="act2", name="act2")
            act3 = cpool.tile([128, 64, BC], adt, tag="act3", name="act3")
            act4 = cpool.tile([128, 16, BC], bf16, tag="act4", name="act4")
            act5 = cpool.tile([128, 2, BC], bf16, tag="act5", name="act5")
            act6 = cpool.tile([128, 2, BC], bf16, tag="act6", name="act6")
            lp_t = cpool.tile([128, 2, BC], f32, tag="lp", name="lp")
            t2 = cpool.tile([128, BC], bf16, tag="t2", name="t2")
            stage = {}
            mask_t = {}
            if fast:
                nc.sync.dma_start(t2[0:NK, :], d["kg"][:])
                mask_t["i"] = cload("imp", [128, 32, NK], bf16)
                mask_t["c"] = cload("cmp", [128, 64, NK], bf16)
            else:
                mask_t["g"] = cload("gmp", [128, 32, NK], bf16)
                mask_t["i"] = cload("imp", [128, 32, NK], bf16)
                mask_t["c"] = cload("cmp", [128, 64, NK], bf16)

            def dense_layer(wdram, K_kt, mgw, MGn, act_in, post, dt, sub, dr):
                jw = mgw // 128
                KCn = K_kt // sub
                step = 2 if dr else 1
                for mg in range(MGn):
                    chunks = []
                    for kc in range(KCn):
                        wt = wpool.tile([128, sub, mgw], dt, tag="wt", name="wt")
                        nc.sync.dma_start(wt[:], wdram[mg * KCn + kc])
                        chunks.append(wt)
                    for j in range(jw):
                        jc = slice(j * 128, (j + 1) * 128)
                        ps = ppool.tile([128, BC], f32, tag="ps", name="ps")
                        for kt in range(0, K_kt, step):
                            c = chunks[kt // sub]
                            t = kt % sub
                            if dr:
                                nc.tensor.matmul(
                                    ps[:], c[:, t:t + 2, jc], act_in[:, kt:kt + 2, :],
                                    start=(kt == 0), stop=(kt == K_kt - 2),
                                    perf_mode=DR)
                            else:
                                nc.tensor.matmul(
                                    ps[:], c[:, t, jc], act_in[:, kt, :],
                                    start=(kt == 0), stop=(kt == K_kt - 1))
                        post(mg * jw + j, ps)

            def kept(mask, K_kt, act_in, row0):
                kp = spool.tile([128, BC], f32, tag="kp", name="kp")
                for kt in range(K_kt):
                    nc.tensor.matmul(kp[0:NK, :], mask[:, kt, :], act_in[:, kt, :],
                                     start=(kt == 0), stop=(kt == K_kt - 1))
                nc.scalar.copy(t2[row0:row0 + NK, :], kp[0:NK, :])

            def mix_post(bias, scale, avec, cvec, mixd, act_out, kts, skey):
                kts = set(kts)

                def post(m, ps):
                    x1f = fpool.tile([128, BC], f32, tag="x1f", name="x1f")
                    if scale is None:
                        nc.scalar.activation(x1f[:], ps[:], SIG, bias=bias[:, m:m + 1])
                    else:
                        nc.scalar.activation(x1f[:], ps[:], SIG, bias=bias[:, m:m + 1],
                                             scale=scale[:, m:m + 1])
                    mx = ivpool.tile([128, BC], bf16, tag="mx", name="mx")
                    nc.sync.dma_start(mx[:], mixd[:, m, :])
                    tmp = fpool.tile([128, BC], f32, tag="tmp", name="tmp")
                    nc.vector.tensor_scalar_mul(tmp[:], mx[:], avec[:, m:m + 1])
                    if fast:
                        mixf = fpool.tile([128, BC], f32, tag="mixf", name="mixf")
                        nc.vector.scalar_tensor_tensor(
                            mixf[:], x1f[:], cvec[:, m:m + 1], tmp[:],
                            AluOpType.mult, AluOpType.add)
                        nc.vector.tensor_copy(act_out[:, m, :], mixf[:])
                        if m in kts:
                            # full-precision (bf16) stash of this k-tile for
                            # the kept-selection matmul
                            st = cpool.tile([128, BC], bf16, tag=f"{skey}{m}",
                                            name=f"{skey}{m}")
                            nc.vector.tensor_copy(st[:], mixf[:])
                            stage[(skey, m)] = st
                    else:
                        nc.vector.scalar_tensor_tensor(
                            act_out[:, m, :], x1f[:], cvec[:, m:m + 1], tmp[:],
                            AluOpType.mult, AluOpType.add)
                return post

            def kept_staged(mask, kts, skey, row0):
                kp = spool.tile([128, BC], f32, tag="kp", name="kp")
                for i, kt in enumerate(kts):
                    nc.tensor.matmul(kp[0:NK, :], mask[:, kt, :],
                                     stage[(skey, kt)][:],
                                     start=(i == 0), stop=(i == len(kts) - 1))
                nc.scalar.copy(t2[row0:row0 + NK, :], kp[0:NK, :])

            s1 = vt.get("s1t")
            s2 = vt.get("s2t")
            s3 = vt.get("s3t")

            # ---- layer 1: [IN] -> [IN], mix with x_invmea ----
            dense_layer(d["w1p"], 32, 512, 8, act1,
                        mix_post(vt["b1t"], s1, vt["a1t"], vt["c1t"], d["iv"],
                                 act2, inv_kts, "si"),
                        adt, wsub, fast)
            if fast:
                kept_staged(mask_t["i"], inv_kts, "si", NK)
            else:
                kept(mask_t["g"], 32, act1, 0)
                kept(mask_t["i"], 32, act2, NK)

            # ---- layer 2: [IN] -> [ED], mix with x_curv ----
            dense_layer(d["w2p"], 32, 512, 16, act2,
                        mix_post(vt["b2t"], s2, vt["a2t"], vt["c2t"], d["cv"],
                                 act3, curv_kts, "sc"),
                        adt, wsub, fast)
            if fast:
                kept_staged(mask_t["c"], curv_kts, "sc", 2 * NK)
            else:
                kept(mask_t["c"], 64, act3, 2 * NK)

            # ---- layer 3: [ED] -> [PW], scale by mp3 ----
            def post3(m, ps):
                x1f = fpool.tile([128, BC], f32, tag="x1f", name="x1f")
                if fast:
                    nc.scalar.activation(x1f[:], ps[:], SIG,
                                         bias=vt["b3t"][:, m:m + 1],
                                         scale=s3[:, m:m + 1])
                else:
                    nc.scalar.activation(x1f[:], ps[:], SIG,
                                         bias=vt["b3t"][:, m:m + 1])
                nc.vector.tensor_scalar_mul(act4[:, m, :], x1f[:],
                                            vt["mp3t"][:, m:m + 1])
            dense_layer(d["w3p"], 64, 512, 4, act3, post3, adt, wsub, fast)
            kept(pm, 16, act4, 3 * NK)

            # ---- layer 4: [PW] -> [OUT] ----
            def post4(m, ps):
                nc.scalar.activation(act5[:, m, :], ps[:], SIG,
                                     bias=vt["b4t"][:, m:m + 1])
            dense_layer(d["w4p"], 16, 256, 1, act4, post4, bf16, 8, False)

            # ---- layer 5: [OUT] -> [OUT] ----
            for j in range(2):
                ps = ppool.tile([128, BC], f32, tag="ps", name="ps")
                for kt in range(2):
                    nc.tensor.matmul(ps[:], w5t[:, kt, j * 128:(j + 1) * 128],
                                     act5[:, kt, :], start=(kt == 0), stop=(kt == 1))
                nc.scalar.activation(act6[:, j, :], ps[:], SIG,
                                     bias=vt["b5t"][:, j:j + 1])

            # ---- layer 6: x_cat [400] -> lp [OUT] ----
            for j in range(2):
                jc = slice(j * 128, (j + 1) * 128)
                ps = ppool.tile([128, BC], f32, tag="ps", name="ps")
                nc.tensor.matmul(ps[:], w6a[:, 0, jc], act6[:, 0, :],
                                 start=True, stop=False)
                nc.tensor.matmul(ps[:], w6a[:, 1, jc], act6[:, 1, :],
                                 start=False, stop=False)
                nc.tensor.matmul(ps[:], w6a[:, 2, jc], t2[:],
                                 start=False, stop=False)
                nc.tensor.matmul(ps[:], w6b[:, jc], cl_t[:],
                                 start=False, stop=True)
                nc.scalar.activation(lp_t[:, j, :], ps[:], SIG)

            # ---- final: out = w7c @ lp (fp32, mean-centering folded in) ----
            fps = spool.tile([128, BC], f32, tag="kp", name="fps")
            nc.tensor.matmul(fps[0:1, :], w7t[:, 0:1], lp_t[:, 0, :],
                             start=True, stop=False)
            nc.tensor.matmul(fps[0:1, :], w7t[:, 1:2], lp_t[:, 1, :],
                             start=False, stop=True)
            osb = cpool.tile([1, BC], f32, tag="osb", name="osb")
            nc.scalar.copy(osb[:], fps[0:1, :])
            nc.sync.dma_start(yd[:], osb[:])

    nc.compile()
    _prog_cache[key] = nc
    return nc


def _host_prep(inputs, fast):
    m1 = (inputs["W1"] * inputs["Adj"]).astype(F32)
    m2 = (inputs["W2"] * inputs["edge_mask"]).astype(F32)
    m3 = (inputs["W3"] * inputs["pathway_mask"]).astype(F32)
    w4t = np.ascontiguousarray(inputs["W4"].T).astype(BF)
    w5T = np.ascontiguousarray(inputs["W5"].T).astype(BF)
    w6T = np.ascontiguousarray(inputs["W6"].T).astype(BF)  # [400, 256]
    w7c = (inputs["W7"][0] - inputs["W7"].sum() / OUT).astype(F32)

    shared = {
        "w4p": _pack_w(w4t, 256, 8),
        "w5t": np.ascontiguousarray(w5T.reshape(2, 128, OUT).transpose(1, 0, 2)),
        "w6a": np.ascontiguousarray(w6T[:384].reshape(3, 128, OUT).transpose(1, 0, 2)),
        "w6b": np.ascontiguousarray(w6T[384:400]),
        "w7ct": _pack_vec(w7c),
        "b1t": _pack_vec(inputs["b1"]),
        "a1t": _pack_vec(inputs["mp11"] * inputs["mp1"]),
        "c1t": _pack_vec(inputs["mp12"] * inputs["mp1"]),
        "b2t": _pack_vec(inputs["b2"]),
        "a2t": _pack_vec(inputs["mp21"] * inputs["mp2"]),
        "c2t": _pack_vec(inputs["mp22"] * inputs["mp2"]),
        "b3t": _pack_vec(inputs["b3"]),
        "mp3t": _pack_vec(inputs["mp3"]),
        "b4t": _pack_vec(inputs["b4"]),
        "b5t": _pack_vec(inputs["b5"]),
        "pmp": _pack_mask(inputs["top_path_mask"]),
        "imp": _pack_mask(inputs["top_invmea_mask"]),
        "cmp": _pack_mask(inputs["top_curv_mask"]),
    }
    if fast:
        s1, q1t = _rowscale_fp8(m1)
        s2, q2t = _rowscale_fp8(m2)
        s3, q3t = _rowscale_fp8(m3)
        shared.update({
            "w1p": _pack_w(q1t, 512, 16),
            "w2p": _pack_w(q2t, 512, 16),
            "w3p": _pack_w(q3t, 512, 16),
            "s1t": _pack_vec(s1),
            "s2t": _pack_vec(s2),
            "s3t": _pack_vec(s3),
        })
    else:
        shared.update({
            "w1p": _pack_w(np.ascontiguousarray(m1.T).astype(BF), 512, 8),
            "w2p": _pack_w(np.ascontiguousarray(m2.T).astype(BF), 512, 8),
            "w3p": _pack_w(np.ascontiguousarray(m3.T).astype(BF), 512, 8),
            "gmp": _pack_mask(inputs["top_gene_mask"]),
        })
    return shared


def kernel(**inputs):
    inputs = {k: np.asarray(v) for k, v in inputs.items()}

    # fast path requires: masked weights exactly fp8-representable after
    # row normalization, and one-hot top_* selection masks.
    s1, _ = _rowscale_fp8((inputs["W1"] * inputs["Adj"]).astype(F32))
    s2, _ = _rowscale_fp8((inputs["W2"] * inputs["edge_mask"]).astype(F32))
    s3, _ = _rowscale_fp8((inputs["W3"] * inputs["pathway_mask"]).astype(F32))
    iidx = _onehot_idx(np.asarray(inputs["top_invmea_mask"], F32))
    cidx = _onehot_idx(np.asarray(inputs["top_curv_mask"], F32))
    fast = all(x is not None for x in (s1, s2, s3, iidx, cidx))

    if fast:
        nc = _build_program("fast", iidx, cidx)
    else:
        nc = _build_program("safe")
    shared = _host_prep(inputs, fast)
    adt = F8 if fast else BF

    in_maps = []
    for c in range(NCORES):
        s = slice(c * BC, (c + 1) * BC)
        m = dict(shared)
        m["xg"] = _pack_act(inputs["x_gene"][s].T.astype(adt), adt)
        m["iv"] = _pack_act(inputs["x_invmea"][s].T.astype(BF), BF)
        m["cv"] = _pack_act(inputs["x_curv"][s].T.astype(BF), BF)
        m["cl"] = np.ascontiguousarray(inputs["clinn"][s].T).astype(BF)
        if fast:
            kg = inputs["x_gene"][s].astype(F32) @ inputs["top_gene_mask"].astype(F32)
            m["kg"] = np.ascontiguousarray(kg.T).astype(BF)
        in_maps.append(m)

    from concourse.bass_utils import run_bass_kernel_spmd

    kwargs = {}
    if TRACE:
        import sys, types
        try:
            from trn_agent_boot.trn_boot import _ntff_profile_via_ctypes
            hook = _ntff_profile_via_ctypes("/opt/axon/libaxon_pjrt.so")
            if hook is not None:
                mod = types.ModuleType("antenv.axon_hooks")
                mod.get_axon_ntff_profile_hook = lambda: hook
                sys.modules["antenv.axon_hooks"] = mod
                import concourse.bass_utils as _bu
                _bu.upload_artifacts = lambda tmpdir: "local://" + tmpdir
                kwargs["trace"] = True
                if TRACE_DIR:
                    kwargs["tmpdir"] = TRACE_DIR
        except Exception as e:
            print("trace setup failed:", e)

    res = run_bass_kernel_spmd(nc, in_maps, core_ids=list(range(NCORES)), **kwargs)
    if TRACE:
        kernel.last_exec_time_ns = res.exec_time_ns

    out = np.concatenate(
        [res.results[c]["y"].reshape(BC, 1) for c in range(NCORES)], axis=0
    )
    return out.astype(F32)


# revision 19
# speedup vs baseline: 1.7878x; 1.1331x over previous
"""Trainium2 Bass kernel for nn_Curv_Net (masked-MLP / GNN message passing).

Sharding: data-parallel over the batch dim across 8 NeuronCores (256 rows
each).  All masked weights (W*mask) are prepared on the host: transposed to
[K, M], row-normalized and cast to fp8-e4m3 when that is exact (it is for
the reference's constant-fill W1/W2/W3: the masked weight is scale*mask),
otherwise bf16.  On device everything flows in a transposed activation
layout actT[feature, batch]; each dense layer runs PE matmuls with the
weight tile stationary and the activation tile moving (N=256), accumulating
K in PSUM.  The three big layers use fp8 DoubleRow (2 contraction rows per
cycle -> 2x PE throughput); the per-row weight scale is folded into the
sigmoid's scale operand.  The stop-gradient "kept" bypass values are kept
at full precision: kept_gene is computed on the host (pure input
selection), kept_invmea/kept_curv are row-gathered by DMA from the f32
mixed activations before the fp8 cast, and kept_path stays on the bf16
path.  The final mean-centering is folded into W7 on the host:
(lp - mean(lp)) @ W7.T == lp @ (W7 - sum(W7)/OUT).T exactly.
"""

import numpy as np
import ml_dtypes

B, IN, ED, PW, OUT, CL, NK = 2048, 4096, 8192, 2048, 256, 16, 32
NCORES = 8
BC = B // NCORES  # 256 batch rows per core

BF = ml_dtypes.bfloat16
F8 = ml_dtypes.float8_e4m3
F32 = np.float32

TRACE = False
TRACE_DIR = None

_prog_cache = {}


def _pack_w(wT, mgw, sub):
    """wT [K, M] -> [MGn*KCn, 128, sub, mgw] chunk-contiguous.

    chunk (mg, kc) holds rows kc*sub*128..+sub*128, cols mg*mgw..+mgw with
    layout [p, t, m] = wT[kc*sub*128 + t*128 + p, mg*mgw + m].
    """
    K, M = wT.shape
    KCn = K // (sub * 128)
    MGn = M // mgw
    a = wT.reshape(KCn, sub, 128, MGn, mgw).transpose(3, 0, 2, 1, 4)
    return np.ascontiguousarray(a).reshape(MGn * KCn, 128, sub, mgw)


def _pack_act(xT, dtype):
    """xT [K, BC] -> [128, K/128, BC] p-major contiguous."""
    K = xT.shape[0]
    a = xT.reshape(K // 128, 128, xT.shape[1]).transpose(1, 0, 2)
    return np.ascontiguousarray(a).astype(dtype)


def _pack_vec(v):
    """v [n] -> [128, n/128] f32."""
    return np.ascontiguousarray(np.asarray(v, F32).reshape(-1, 128).T).astype(F32)


def _pack_mask(m):
    """mask [K, NK] -> [128, K/128, NK] bf16 p-major."""
    K = m.shape[0]
    a = m.reshape(K // 128, 128, NK).transpose(1, 0, 2)
    return np.ascontiguousarray(a.astype(BF))


def _rowscale_fp8(masked):
    """masked [M, K] -> (scale [M], q [K, M] fp8) with masked == s*q exact,
    or (None, None) if not exactly representable."""
    s = np.abs(masked).max(axis=1)
    s[s == 0] = 1.0
    q = masked / s[:, None]
    q8 = q.astype(F8)
    if not np.array_equal(q8.astype(F32), q):
        return None, None
    return s.astype(F32), np.ascontiguousarray(q8.T)


def _onehot_idx(mask):
    """mask [K, NK] -> row index per column if exactly one-hot, else None."""
    if not np.all((mask == 0) | (mask == 1)):
        return None
    if not np.array_equal(mask.sum(axis=0), np.ones(mask.shape[1], F32)):
        return None
    return np.argmax(mask, axis=0)


def _build_program(mode, iidx=None, cidx=None):
    key = (mode, None if iidx is None else (tuple(iidx), tuple(cidx)))
    if key in _prog_cache:
        return _prog_cache[key]

    import concourse.bacc as bacc
    import concourse.mybir as mybir
    import concourse.tile as tile
    from concourse.alu_op_type import AluOpType

    bf16 = mybir.dt.bfloat16
    fp8 = mybir.dt.float8e4
    f32 = mybir.dt.float32
    SIG = mybir.ActivationFunctionType.Sigmoid
    DR = mybir.MatmulPerfMode.DoubleRow
    fast = mode == "fast"
    adt = fp8 if fast else bf16           # dtype of the big-layer activations
    wsub = 16 if fast else 8              # k-subtiles per big-layer chunk

    nc = bacc.Bacc("TRN2", target_bir_lowering=False, debug=False)

    # ---- DRAM I/O -------------------------------------------------------
    d = {}
    d["xg"] = nc.dram_tensor("xg", [128, IN // 128, BC], adt, kind="ExternalInput")
    d["iv"] = nc.dram_tensor("iv", [128, IN // 128, BC], bf16, kind="ExternalInput")
    d["cv"] = nc.dram_tensor("cv", [128, ED // 128, BC], bf16, kind="ExternalInput")
    d["cl"] = nc.dram_tensor("cl", [CL, BC], bf16, kind="ExternalInput")
    d["w1p"] = nc.dram_tensor("w1p", [(IN // (wsub * 128)) * (IN // 512), 128, wsub, 512], adt, kind="ExternalInput")
    d["w2p"] = nc.dram_tensor("w2p", [(IN // (wsub * 128)) * (ED // 512), 128, wsub, 512], adt, kind="ExternalInput")
    d["w3p"] = nc.dram_tensor("w3p", [(ED // (wsub * 128)) * (PW // 512), 128, wsub, 512], adt, kind="ExternalInput")
    d["w4p"] = nc.dram_tensor("w4p", [2, 128, 8, 256], bf16, kind="ExternalInput")
    d["w5t"] = nc.dram_tensor("w5t", [128, 2, OUT], bf16, kind="ExternalInput")
    d["w6a"] = nc.dram_tensor("w6a", [128, 3, OUT], bf16, kind="ExternalInput")
    d["w6b"] = nc.dram_tensor("w6b", [CL, OUT], bf16, kind="ExternalInput")
    d["w7ct"] = nc.dram_tensor("w7ct", [128, 2], f32, kind="ExternalInput")
    vec_specs = [("b1t", 32), ("a1t", 32), ("c1t", 32),
                 ("b2t", 64), ("a2t", 64), ("c2t", 64),
                 ("b3t", 16), ("mp3t", 16), ("b4t", 2), ("b5t", 2)]
    if fast:
        vec_specs += [("s1t", 32), ("s2t", 64), ("s3t", 16)]
    for name, n in vec_specs:
        d[name] = nc.dram_tensor(name, [128, n], f32, kind="ExternalInput")
    d["pmp"] = nc.dram_tensor("pmp", [128, 16, NK], bf16, kind="ExternalInput")
    d["imp"] = nc.dram_tensor("imp", [128, 32, NK], bf16, kind="ExternalInput")
    d["cmp"] = nc.dram_tensor("cmp", [128, 64, NK], bf16, kind="ExternalInput")
    if fast:
        d["kg"] = nc.dram_tensor("kg", [NK, BC], bf16, kind="ExternalInput")
    else:
        d["gmp"] = nc.dram_tensor("gmp", [128, 32, NK], bf16, kind="ExternalInput")
    yd = nc.dram_tensor("y", [1, BC], f32, kind="ExternalOutput")

    # k-tiles containing at least one kept-selection row
    inv_kts = sorted({idx // 128 for idx in iidx}) if fast else []
    curv_kts = sorted({idx // 128 for idx in cidx}) if fast else []

    with tile.TileContext(nc) as tc:
        with (
            tc.tile_pool(name="const", bufs=1) as cpool,
            tc.tile_pool(name="wstream", bufs=10) as wpool,
            tc.tile_pool(name="fwork", bufs=4) as fpool,
            tc.tile_pool(name="mixin", bufs=8) as ivpool,
            tc.tile_pool(name="psum_mm", bufs=5, space="PSUM") as ppool,
            tc.tile_pool(name="psum_sm", bufs=2, space="PSUM") as spool,
        ):
            def cload(name, shape, dtype, eng=None):
                t = cpool.tile(shape, dtype, tag=name, name=name + "_sb")
                (eng or nc.scalar).dma_start(t[:], d[name][:])
                return t

            # layer-1 critical data goes first, on the sync (SP) DMA ring;
            # everything else loads on the scalar ring so it never delays
            # the weight stream.
            act1 = cload("xg", [128, 32, BC], adt, eng=nc.sync)
            cl_t = cload("cl", [CL, BC], bf16)
            pm = cload("pmp", [128, 16, NK], bf16)
            w5t = cload("w5t", [128, 2, OUT], bf16)
            w6a = cload("w6a", [128, 3, OUT], bf16)
            w6b = cload("w6b", [CL, OUT], bf16)
            w7t = cload("w7ct", [128, 2], f32)
            vt = {}
            for name, n in vec_specs:
                vt[name] = cload(name, [128, n], f32)

            act2 = cpool.tile([128, 32, BC], adt, tag="act2", name="act2")
            act3 = cpool.tile([128, 64, BC], adt, tag="act3", name="act3")
            act4 = cpool.tile([128, 16, BC], bf16, tag="act4", name="act4")
            act5 = cpool.tile([128, 2, BC], bf16, tag="act5", name="act5")
            act6 = cpool.tile([128, 2, BC], bf16, tag="act6", name="act6")
            lp_t = cpool.tile([128, 2, BC], f32, tag="lp", name="lp")
            t2 = cpool.tile([128, BC], bf16, tag="t2", name="t2")
            stage = {}
            mask_t = {}
            if fast:
                nc.sync.dma_start(t2[0:NK, :], d["kg"][:])
                mask_t["i"] = cload("imp", [128, 32, NK], bf16)
                mask_t["c"] = cload("cmp", [128, 64, NK], bf16)
            else:
                mask_t["g"] = cload("gmp", [128, 32, NK], bf16)
                mask_t["i"] = cload("imp", [128, 32, NK], bf16)
                mask_t["c"] = cload("cmp", [128, 64, NK], bf16)

            def dense_layer(wdram, K_kt, mgw, MGn, act_in, post, dt, sub, dr,
                            pre=None):
                jw = mgw // 128
                KCn = K_kt // sub
                step = 2 if dr else 1
                for mg in range(MGn):
                    if pre is not None:
                        pre(mg)
                    chunks = []
                    for kc in range(KCn):
                        wt = wpool.tile([128, sub, mgw], dt, tag="wt", name="wt")
                        nc.sync.dma_start(wt[:], wdram[mg * KCn + kc])
                        chunks.append(wt)
                    for j in range(jw):
                        jc = slice(j * 128, (j + 1) * 128)
                        ps = ppool.tile([128, BC], f32, tag="ps", name="ps")
                        for kt in range(0, K_kt, step):
                            c = chunks[kt // sub]
                            t = kt % sub
                            if dr:
                                nc.tensor.matmul(
                                    ps[:], c[:, t:t + 2, jc], act_in[:, kt:kt + 2, :],
                                    start=(kt == 0), stop=(kt == K_kt - 2),
                                    perf_mode=DR)
                            else:
                                nc.tensor.matmul(
                                    ps[:], c[:, t, jc], act_in[:, kt, :],
                                    start=(kt == 0), stop=(kt == K_kt - 1))
                        post(mg * jw + j, ps)

            def kept(mask, K_kt, act_in, row0):
                kp = spool.tile([128, BC], f32, tag="kp", name="kp")
                for kt in range(K_kt):
                    nc.tensor.matmul(kp[0:NK, :], mask[:, kt, :], act_in[:, kt, :],
                                     start=(kt == 0), stop=(kt == K_kt - 1))
                nc.scalar.copy(t2[row0:row0 + NK, :], kp[0:NK, :])

            def mix_post(bias, scale, avec, cvec, mixd, act_out, kts, skey,
                         jw=4):
                kts = set(kts)
                strips = {}

                def pre(mg):
                    st = ivpool.tile([128, jw, BC], bf16, tag="mx", name="mx")
                    nc.scalar.dma_start(st[:], mixd[:, mg * jw:(mg + 1) * jw, :])
                    strips[mg] = st

                def post(m, ps):
                    x1f = fpool.tile([128, BC], f32, tag="x1f", name="x1f")
                    if scale is None:
                        nc.scalar.activation(x1f[:], ps[:], SIG, bias=bias[:, m:m + 1])
                    else:
                        nc.scalar.activation(x1f[:], ps[:], SIG, bias=bias[:, m:m + 1],
                                             scale=scale[:, m:m + 1])
                    mx = strips[m // jw][:, m % jw, :]
                    tmp = fpool.tile([128, BC], f32, tag="tmp", name="tmp")
                    nc.vector.tensor_scalar_mul(tmp[:], mx[:], avec[:, m:m + 1])
                    if fast:
                        mixf = fpool.tile([128, BC], f32, tag="mixf", name="mixf")
                        nc.vector.scalar_tensor_tensor(
                            mixf[:], x1f[:], cvec[:, m:m + 1], tmp[:],
                            AluOpType.mult, AluOpType.add)
                        nc.vector.tensor_copy(act_out[:, m, :], mixf[:])
                        if m in kts:
                            # full-precision (bf16) stash of this k-tile for
                            # the kept-selection matmul
                            st = cpool.tile([128, BC], bf16, tag=f"{skey}{m}",
                                            name=f"{skey}{m}")
                            nc.vector.tensor_copy(st[:], mixf[:])
                            stage[(skey, m)] = st
                    else:
                        nc.vector.scalar_tensor_tensor(
                            act_out[:, m, :], x1f[:], cvec[:, m:m + 1], tmp[:],
                            AluOpType.mult, AluOpType.add)
                return pre, post

            def kept_staged(mask, kts, skey, row0):
                kp = spool.tile([128, BC], f32, tag="kp", name="kp")
                for i, kt in enumerate(kts):
                    nc.tensor.matmul(kp[0:NK, :], mask[:, kt, :],
                                     stage[(skey, kt)][:],
                                     start=(i == 0), stop=(i == len(kts) - 1))
                nc.scalar.copy(t2[row0:row0 + NK, :], kp[0:NK, :])

            s1 = vt.get("s1t")
            s2 = vt.get("s2t")
            s3 = vt.get("s3t")

            # ---- layer 1: [IN] -> [IN], mix with x_invmea ----
            pre1, post1 = mix_post(vt["b1t"], s1, vt["a1t"], vt["c1t"],
                                   d["iv"], act2, inv_kts, "si")
            dense_layer(d["w1p"], 32, 512, 8, act1, post1, adt, wsub, fast,
                        pre=pre1)
            if fast:
                kept_staged(mask_t["i"], inv_kts, "si", NK)
            else:
                kept(mask_t["g"], 32, act1, 0)
                kept(mask_t["i"], 32, act2, NK)

            # ---- layer 2: [IN] -> [ED], mix with x_curv ----
            pre2, post2 = mix_post(vt["b2t"], s2, vt["a2t"], vt["c2t"],
                                   d["cv"], act3, curv_kts, "sc")
            dense_layer(d["w2p"], 32, 512, 16, act2, post2, adt, wsub, fast,
                        pre=pre2)
            if fast:
                kept_staged(mask_t["c"], curv_kts, "sc", 2 * NK)
            else:
                kept(mask_t["c"], 64, act3, 2 * NK)

            # ---- layer 3: [ED] -> [PW], scale by mp3 ----
            def post3(m, ps):
                x1f = fpool.tile([128, BC], f32, tag="x1f", name="x1f")
                if fast:
                    nc.scalar.activation(x1f[:], ps[:], SIG,
                                         bias=vt["b3t"][:, m:m + 1],
                                         scale=s3[:, m:m + 1])
                else:
                    nc.scalar.activation(x1f[:], ps[:], SIG,
                                         bias=vt["b3t"][:, m:m + 1])
                nc.vector.tensor_scalar_mul(act4[:, m, :], x1f[:],
                                            vt["mp3t"][:, m:m + 1])
            dense_layer(d["w3p"], 64, 512, 4, act3, post3, adt, wsub, fast)
            kept(pm, 16, act4, 3 * NK)

            # ---- layer 4: [PW] -> [OUT] ----
            def post4(m, ps):
                nc.scalar.activation(act5[:, m, :], ps[:], SIG,
                                     bias=vt["b4t"][:, m:m + 1])
            dense_layer(d["w4p"], 16, 256, 1, act4, post4, bf16, 8, False)

            # ---- layer 5: [OUT] -> [OUT] ----
            for j in range(2):
                ps = ppool.tile([128, BC], f32, tag="ps", name="ps")
                for kt in range(2):
                    nc.tensor.matmul(ps[:], w5t[:, kt, j * 128:(j + 1) * 128],
                                     act5[:, kt, :], start=(kt == 0), stop=(kt == 1))
                nc.scalar.activation(act6[:, j, :], ps[:], SIG,
                                     bias=vt["b5t"][:, j:j + 1])

            # ---- layer 6: x_cat [400] -> lp [OUT] ----
            for j in range(2):
                jc = slice(j * 128, (j + 1) * 128)
                ps = ppool.tile([128, BC], f32, tag="ps", name="ps")
                nc.tensor.matmul(ps[:], w6a[:, 0, jc], act6[:, 0, :],
                                 start=True, stop=False)
                nc.tensor.matmul(ps[:], w6a[:, 1, jc], act6[:, 1, :],
                                 start=False, stop=False)
                nc.tensor.matmul(ps[:], w6a[:, 2, jc], t2[:],
                                 start=False, stop=False)
                nc.tensor.matmul(ps[:], w6b[:, jc], cl_t[:],
                                 start=False, stop=True)
                nc.scalar.activation(lp_t[:, j, :], ps[:], SIG)

            # ---- final: out = w7c @ lp (fp32, mean-centering folded in) ----
            fps = spool.tile([128, BC], f32, tag="kp", name="fps")
            nc.tensor.matmul(fps[0:1, :], w7t[:, 0:1], lp_t[:, 0, :],
                             start=True, stop=False)
            nc.tensor.matmul(fps[0:1, :], w7t[:, 1:2], lp_t[:, 1, :],
                             start=False, stop=True)
            osb = cpool.tile([1, BC], f32, tag="osb", name="osb")
            nc.scalar.copy(osb[:], fps[0:1, :])
            nc.sync.dma_start(yd[:], osb[:])

    nc.compile()
    _prog_cache[key] = nc
    return nc


def _host_prep(inputs, fast):
    m1 = (inputs["W1"] * inputs["Adj"]).astype(F32)
    m2 = (inputs["W2"] * inputs["edge_mask"]).astype(F32)
    m3 = (inputs["W3"] * inputs["pathway_mask"]).astype(F32)
    w4t = np.ascontiguousarray(inputs["W4"].T).astype(BF)
    w5T = np.ascontiguousarray(inputs["W5"].T).astype(BF)
    w6T = np.ascontiguousarray(inputs["W6"].T).astype(BF)  # [400, 256]
    w7c = (inputs["W7"][0] - inputs["W7"].sum() / OUT).astype(F32)

    shared = {
        "w4p": _pack_w(w4t, 256, 8),
        "w5t": np.ascontiguousarray(w5T.reshape(2, 128, OUT).transpose(1, 0, 2)),
        "w6a": np.ascontiguousarray(w6T[:384].reshape(3, 128, OUT).transpose(1, 0, 2)),
        "w6b": np.ascontiguousarray(w6T[384:400]),
        "w7ct": _pack_vec(w7c),
        "b1t": _pack_vec(inputs["b1"]),
        "a1t": _pack_vec(inputs["mp11"] * inputs["mp1"]),
        "c1t": _pack_vec(inputs["mp12"] * inputs["mp1"]),
        "b2t": _pack_vec(inputs["b2"]),
        "a2t": _pack_vec(inputs["mp21"] * inputs["mp2"]),
        "c2t": _pack_vec(inputs["mp22"] * inputs["mp2"]),
        "b3t": _pack_vec(inputs["b3"]),
        "mp3t": _pack_vec(inputs["mp3"]),
        "b4t": _pack_vec(inputs["b4"]),
        "b5t": _pack_vec(inputs["b5"]),
        "pmp": _pack_mask(inputs["top_path_mask"]),
        "imp": _pack_mask(inputs["top_invmea_mask"]),
        "cmp": _pack_mask(inputs["top_curv_mask"]),
    }
    if fast:
        s1, q1t = _rowscale_fp8(m1)
        s2, q2t = _rowscale_fp8(m2)
        s3, q3t = _rowscale_fp8(m3)
        shared.update({
            "w1p": _pack_w(q1t, 512, 16),
            "w2p": _pack_w(q2t, 512, 16),
            "w3p": _pack_w(q3t, 512, 16),
            "s1t": _pack_vec(s1),
            "s2t": _pack_vec(s2),
            "s3t": _pack_vec(s3),
        })
    else:
        shared.update({
            "w1p": _pack_w(np.ascontiguousarray(m1.T).astype(BF), 512, 8),
            "w2p": _pack_w(np.ascontiguousarray(m2.T).astype(BF), 512, 8),
            "w3p": _pack_w(np.ascontiguousarray(m3.T).astype(BF), 512, 8),
            "gmp": _pack_mask(inputs["top_gene_mask"]),
        })
    return shared


def kernel(**inputs):
    inputs = {k: np.asarray(v) for k, v in inputs.items()}

    # fast path requires: masked weights exactly fp8-representable after
    # row normalization, and one-hot top_* selection masks.
    s1, _ = _rowscale_fp8((inputs["W1"] * inputs["Adj"]).astype(F32))
    s2, _ = _rowscale_fp8((inputs["W2"] * inputs["edge_mask"]).astype(F32))
    s3, _ = _rowscale_fp8((inputs["W3"] * inputs["pathway_mask"]).astype(F32))
    iidx = _onehot_idx(np.asarray(inputs["top_invmea_mask"], F32))
    cidx = _onehot_idx(np.asarray(inputs["top_curv_mask"], F32))
    fast = all(x is not None for x in (s1, s2, s3, iidx, cidx))

    if fast:
        nc = _build_program("fast", iidx, cidx)
    else:
        nc = _build_program("safe")
    shared = _host_prep(inputs, fast)
    adt = F8 if fast else BF

    in_maps = []
    for c in range(NCORES):
        s = slice(c * BC, (c + 1) * BC)
        m = dict(shared)
        m["xg"] = _pack_act(inputs["x_gene"][s].T.astype(adt), adt)
        m["iv"] = _pack_act(inputs["x_invmea"][s].T.astype(BF), BF)
        m["cv"] = _pack_act(inputs["x_curv"][s].T.astype(BF), BF)
        m["cl"] = np.ascontiguousarray(inputs["clinn"][s].T).astype(BF)
        if fast:
            kg = inputs["x_gene"][s].astype(F32) @ inputs["top_gene_mask"].astype(F32)
            m["kg"] = np.ascontiguousarray(kg.T).astype(BF)
        in_maps.append(m)

    from concourse.bass_utils import run_bass_kernel_spmd

    kwargs = {}
    if TRACE:
        import sys, types
        try:
            from trn_agent_boot.trn_boot import _ntff_profile_via_ctypes
            hook = _ntff_profile_via_ctypes("/opt/axon/libaxon_pjrt.so")
            if hook is not None:
                mod = types.ModuleType("antenv.axon_hooks")
                mod.get_axon_ntff_profile_hook = lambda: hook
                sys.modules["antenv.axon_hooks"] = mod
                import concourse.bass_utils as _bu
                _bu.upload_artifacts = lambda tmpdir: "local://" + tmpdir
                kwargs["trace"] = True
                if TRACE_DIR:
                    kwargs["tmpdir"] = TRACE_DIR
        except Exception as e:
            print("trace setup failed:", e)

    res = run_bass_kernel_spmd(nc, in_maps, core_ids=list(range(NCORES)), **kwargs)
    if TRACE:
        kernel.last_exec_time_ns = res.exec_time_ns

    out = np.concatenate(
        [res.results[c]["y"].reshape(BC, 1) for c in range(NCORES)], axis=0
    )
    return out.astype(F32)
